# revision 1
# baseline (speedup 1.0000x reference)
"""GroupedQueryAttention TRN2 Bass kernel.

Strategy (8 NeuronCores, tensor-parallel over heads):
  - Each core owns 4 q-heads (one kv head, GQA group of 4).
  - Host pre-transposes x and the weight slices so every matmul operand
    already has its contraction dim on SBUF partitions.
  - Per core: QKV projection (fp32r matmuls), fused RoPE (DVE),
    causal flash-style attention per (batch, head, q-block):
      S^T = K^T.T @ Q^T  -> exp on ACT -> causal mask via gpsimd
      ctx^T = V_aug.T @ exp  (V augmented with a ones column so the
      softmax denominator falls out of the same matmul)
  - AllGather of ctx^T (E/8 rows per core) across the 8 cores, then each
    core computes a 256-column slice of the output projection.
  - Host concatenates + transposes the 8 slices into the full output.
"""

import os
import sys

import numpy as np


def _ensure_concourse():
    try:
        import concourse.bass  # noqa: F401
    except ImportError:
        for p in ("/opt/trn_rl_repo", "/root/.axon_site/_ro/trn_rl_repo"):
            if os.path.isdir(p) and p not in sys.path:
                sys.path.insert(0, p)
        import concourse.bass  # noqa: F401


FULL_CFG = dict(B=2, S=2048, E=2048, NH=32, NKV=8, HD=64, ncores=8, IC=256, IC2=512)

LAST_RESULTS = None  # BassKernelResults of the most recent kernel() call


def build_gqa(cfg):
    """Build the Bass module for one core's SPMD program. Returns nc."""
    _ensure_concourse()
    from contextlib import ExitStack

    import concourse.mybir as mybir
    import concourse.tile as tile
    from concourse import bacc
    from concourse.masks import make_identity

    dt = mybir.dt
    f32 = dt.float32
    f32r = dt.bfloat16 if cfg.get("mmdt", "bf16") == "bf16" else dt.float32r
    Exp = mybir.ActivationFunctionType.Exp

    B, S, E = cfg["B"], cfg["S"], cfg["E"]
    NH, NKV, HD = cfg["NH"], cfg["NKV"], cfg["HD"]
    NCORES = cfg["ncores"]
    HPC = NH // NCORES          # q heads per core
    assert HPC == 4 and HD == 64
    QH = HPC * HD               # 256: per-core q/ctx/out rows
    KVD = 2 * HD                # 128: packed K|V projection width
    NI = B * S                  # total tokens
    ET = E // 128               # contraction tiles
    IC = cfg["IC"]              # phase-1 token chunk
    IC2 = cfg["IC2"]            # phase-5 token chunk
    QB = 512                    # attention q block
    KB = 128                    # attention k block
    NQT = S // QB               # q blocks per batch
    SKT = S // KB               # k blocks per batch
    NKTILES = NI // KB          # total k tiles (both batches)
    scale = 1.0 / float(np.sqrt(HD))

    nc = bacc.Bacc("TRN2", target_bir_lowering=False, debug=False,
                   num_devices=NCORES)

    xT = nc.dram_tensor("xT", [E, NI], f32r, kind="ExternalInput").ap()
    wqT = nc.dram_tensor("wqT", [E, QH], f32r, kind="ExternalInput").ap()
    wkvT = nc.dram_tensor("wkvT", [E, KVD], f32r, kind="ExternalInput").ap()
    woT = nc.dram_tensor("woT", [E, QH], f32r, kind="ExternalInput").ap()
    cosT = nc.dram_tensor("cosT", [128, S], f32, kind="ExternalInput").ap()
    sinT = nc.dram_tensor("sinT", [128, S], f32, kind="ExternalInput").ap()
    outT = nc.dram_tensor("outT", [QH, NI], f32, kind="ExternalOutput").ap()

    with tile.TileContext(nc) as tc, ExitStack() as persist:
        ctxsb_pool = persist.enter_context(tc.tile_pool(name="ctxsb", bufs=1))
        proj_ps = persist.enter_context(
            tc.tile_pool(name="proj_ps", bufs=2, space="PSUM"))
        scores_ps = persist.enter_context(
            tc.tile_pool(name="scores_ps", bufs=2, space="PSUM"))
        ctx_ps_pool = persist.enter_context(
            tc.tile_pool(name="ctx_ps", bufs=2, space="PSUM"))
        dram = persist.enter_context(
            tc.tile_pool(name="dram", bufs=1, space="DRAM"))
        ph13 = persist.enter_context(ExitStack())
        const = ph13.enter_context(tc.tile_pool(name="const", bufs=1))
        qt_pool = ph13.enter_context(tc.tile_pool(name="qt", bufs=1))
        kt_pool = ph13.enter_context(tc.tile_pool(name="kt", bufs=1))
        vaug_pool = ph13.enter_context(tc.tile_pool(name="vaug", bufs=1))

        ident = const.tile([128, 128], f32, name="ident", tag="ident")
        make_identity(nc, ident[:, :])
        cos_sb = const.tile([128, S], f32, name="cos_sb", tag="cos")
        nc.sync.dma_start(cos_sb[:, :], cosT)
        sin_sb = const.tile([128, S], f32, name="sin_sb", tag="sin")
        nc.sync.dma_start(sin_sb[:, :], sinT)
        wq_sb = const.tile([128, ET, QH], f32r, name="wq_sb", tag="wq")
        nc.sync.dma_start(wq_sb[:, :, :],
                          wqT.rearrange("(t p) o -> p t o", p=128))
        wkv_sb = const.tile([128, ET, KVD], f32r, name="wkv_sb", tag="wkv")
        nc.sync.dma_start(wkv_sb[:, :, :],
                          wkvT.rearrange("(t p) o -> p t o", p=128))
        ones_col = const.tile([128, 1], f32, name="ones_col", tag="ones")
        nc.gpsimd.memset(ones_col[:, :], 1.0)
        nqb = QB // KB
        mask_sb = const.tile([128, nqb, QB], f32r, name="mask_sb", tag="mask")
        nc.gpsimd.memset(mask_sb[:, :, :], 1.0)
        for j in range(nqb):
            # keep where dq >= dk + KB*j, else 0 (causal within diagonal band)
            nc.gpsimd.affine_select(
                out=mask_sb[:, j, :], in_=mask_sb[:, j, :],
                pattern=[[1, QB]], compare_op=mybir.AluOpType.is_ge,
                fill=0.0, base=-KB * j, channel_multiplier=-1)

        # persistent activations
        qt_sb = [qt_pool.tile([128, NI], f32r, name=f"qt{m}", tag=f"qt{m}")
                 for m in range(HPC // 2)]
        kt_sb = kt_pool.tile([128, NI], f32r, tag="ktd")  # K^T duplicated 2x
        vaug = [vaug_pool.tile([128, HD + 1], f32r, name=f"va{k}", tag=f"va{k}")
                for k in range(NKTILES)]
        ctx_sb = [ctxsb_pool.tile([128, NI], f32r, name=f"cx{m}", tag=f"cx{m}")
                  for m in range(HPC // 2)]

        def rope(dst, src_ps, parts, s0, ln, qs_t, t1_t):
            # dst = src*cos + swap(src)*signed_sin ; src is PSUM, dst SBUF
            for h0 in range(0, parts, 64):
                nc.vector.tensor_copy(qs_t[h0:h0 + 32, :ln],
                                      src_ps[h0 + 32:h0 + 64, :ln])
                nc.vector.tensor_copy(qs_t[h0 + 32:h0 + 64, :ln],
                                      src_ps[h0:h0 + 32, :ln])
            nc.vector.tensor_mul(t1_t[:parts, :ln], src_ps[:parts, :ln],
                                 cos_sb[:parts, s0:s0 + ln])
            nc.vector.tensor_mul(qs_t[:parts, :ln], qs_t[:parts, :ln],
                                 sin_sb[:parts, s0:s0 + ln])
            nc.vector.tensor_add(dst, t1_t[:parts, :ln], qs_t[:parts, :ln])

        # ---- phase 1-3 scoped pools
        if True:
            xt_pool = ph13.enter_context(tc.tile_pool(name="xt", bufs=2))
            rope_pool = ph13.enter_context(tc.tile_pool(name="rope", bufs=2))
            vs_pool = ph13.enter_context(tc.tile_pool(name="vs", bufs=2))
            exp_pool = ph13.enter_context(tc.tile_pool(name="exp", bufs=3))
            rc_pool = ph13.enter_context(tc.tile_pool(name="rc", bufs=2))
            rb_pool = ph13.enter_context(tc.tile_pool(name="rb", bufs=2))

            # ---- phase 1: QKV projection + RoPE + V transpose
            for ch in range(NI // IC):
                i0 = ch * IC
                s0 = i0 % S
                xt = xt_pool.tile([128, ET, IC], f32r, name="xt", tag="xt")
                nc.sync.dma_start(
                    xt[:, :, :],
                    xT[:, i0:i0 + IC].rearrange("(t p) i -> p t i", p=128))
                for m in range(HPC // 2):
                    q_ps = proj_ps.tile([128, IC], f32, name="pps", tag="proj")
                    for t in range(ET):
                        nc.tensor.matmul(
                            q_ps[:, :],
                            wq_sb[:, t, m * 128:(m + 1) * 128],
                            xt[:, t, :],
                            start=(t == 0), stop=(t == ET - 1))
                    qs_t = rope_pool.tile([128, IC], f32, name="qs_t", tag="qs")
                    t1_t = rope_pool.tile([128, IC], f32, name="t1_t", tag="t1")
                    rope(qt_sb[m][:, i0:i0 + IC], q_ps, 128, s0, IC, qs_t, t1_t)
                kv_ps = proj_ps.tile([128, IC], f32, name="pps", tag="proj")
                for t in range(ET):
                    nc.tensor.matmul(
                        kv_ps[:, :],
                        wkv_sb[:, t, :],
                        xt[:, t, :],
                        start=(t == 0), stop=(t == ET - 1))
                qs_t = rope_pool.tile([128, IC], f32, name="qs_t", tag="qs")
                t1_t = rope_pool.tile([128, IC], f32, name="t1_t", tag="t1")
                rope(kt_sb[0:64, i0:i0 + IC], kv_ps, 64, s0, IC, qs_t, t1_t)
                nc.vector.tensor_copy(kt_sb[64:128, i0:i0 + IC],
                                      kt_sb[0:64, i0:i0 + IC])
                vs = vs_pool.tile([64, IC], f32, name="vs", tag="vs")
                nc.vector.tensor_copy(vs[:, :], kv_ps[64:128, :])
                for j in range(IC // 128):
                    kidx = (i0 + j * 128) // 128
                    vt_ps = scores_ps.tile([128, 2 * QB], f32, name="s_ps", tag="s")
                    nc.tensor.transpose(vt_ps[:, 0:64], vs[:, j * 128:(j + 1) * 128],
                                        ident[0:64, 0:64])
                    nc.vector.tensor_copy(vaug[kidx][:, 0:HD], vt_ps[:, 0:64])
                    nc.vector.tensor_copy(vaug[kidx][:, HD:HD + 1], ones_col[:, :])

            # ---- phase 3: attention
            for b in range(B):
                for qt in range(NQT):
                    for h in range(HPC):
                        mt, hb = h // 2, (h % 2) * 64
                        q_ap = qt_sb[mt][hb:hb + 64, b * S + qt * QB:
                                         b * S + qt * QB + QB]
                        ctx_ps = ctx_ps_pool.tile([128, QB], f32, name="ctx_ps", tag="ctx")
                        nkt = (qt + 1) * (QB // KB)
                        for kp in range(nkt // 2):
                            s_ps = scores_ps.tile([128, 2 * QB], f32, name="s_ps", tag="s")
                            e_t = exp_pool.tile([128, 2 * QB], f32r, name="e_t", tag="e")
                            for hf in range(2):
                                kt = kp * 2 + hf
                                nc.tensor.matmul(
                                    s_ps[:, hf * QB:(hf + 1) * QB],
                                    kt_sb[hb:hb + 64,
                                          b * S + kt * KB:b * S + kt * KB + KB
                                          ],
                                    q_ap,
                                    start=True, stop=True)
                            nc.scalar.activation(e_t[:, :], s_ps[:, :], Exp,
                                                 scale=scale)
                            for hf in range(2):
                                kt = kp * 2 + hf
                                j = kt - qt * (QB // KB)
                                if j >= 0:  # diagonal block: causal mask
                                    nc.vector.tensor_mul(
                                        e_t[:, hf * QB:(hf + 1) * QB],
                                        e_t[:, hf * QB:(hf + 1) * QB],
                                        mask_sb[:, j, :])
                                nc.tensor.matmul(
                                    ctx_ps[0:HD + 1, :],
                                    vaug[b * SKT + kt][:, :],
                                    e_t[:, hf * QB:(hf + 1) * QB],
                                    start=(kt == 0), stop=(kt == nkt - 1))
                        recip = rc_pool.tile([1, QB], f32, name="recip", tag="rc")
                        nc.vector.reciprocal(recip[0:1, :], ctx_ps[HD:HD + 1, :])
                        rbc = rb_pool.tile([64, QB], f32, name="rbc", tag="rb")
                        nc.gpsimd.partition_broadcast(rbc[:, :], recip[0:1, :])
                        nc.vector.tensor_mul(
                            ctx_sb[mt][hb:hb + 64,
                                       b * S + qt * QB:b * S + qt * QB + QB],
                            ctx_ps[0:HD, :], rbc[:, :])

        ph13.close()

        # ---- phase 4: AllGather ctx^T across cores
        cc_in = dram.tile([QH, NI], f32r, name="cc_in", tag="ccin")
        for m in range(HPC // 2):
            nc.sync.dma_start(cc_in[m * 128:(m + 1) * 128, :], ctx_sb[m][:, :])
        cc_out = dram.tile([E, NI], f32r, name="cc_out", tag="ccout",
                           addr_space="Shared" if NCORES > 4 else "Local")
        nc.gpsimd.collective_compute(
            "AllGather", mybir.AluOpType.bypass,
            replica_groups=[list(range(NCORES))],
            ins=[cc_in[:, :]],
            outs=[cc_out[:, :]])

        # ---- phase 5: output projection slice
        with ExitStack() as ph5:
            wo_pool = ph5.enter_context(tc.tile_pool(name="wo", bufs=1))
            ct_pool = ph5.enter_context(tc.tile_pool(name="ct", bufs=2))
            ob_pool = ph5.enter_context(tc.tile_pool(name="ob", bufs=2))
            wo_sb = wo_pool.tile([128, ET, QH], f32r, name="wo_sb", tag="wo")
            nc.sync.dma_start(wo_sb[:, :, :],
                              woT.rearrange("(t p) o -> p t o", p=128))
            for ch in range(NI // IC2):
                i0 = ch * IC2
                ct = ct_pool.tile([128, ET, IC2], f32r, name="ct", tag="ct")
                nc.sync.dma_start(
                    ct[:, :, :],
                    cc_out[:, i0:i0 + IC2].rearrange("(t p) i -> p t i", p=128))
                for m in range(HPC // 2):
                    o_ps = proj_ps.tile([128, IC2], f32, name="ops", tag="proj")
                    for t in range(ET):
                        nc.tensor.matmul(
                            o_ps[:, :],
                            wo_sb[:, t, m * 128:(m + 1) * 128],
                            ct[:, t, :],
                            start=(t == 0), stop=(t == ET - 1))
                    ob = ob_pool.tile([128, IC2], f32, name="ob", tag="ob")
                    nc.vector.tensor_copy(ob[:, :], o_ps[:, :])
                    nc.sync.dma_start(outT[m * 128:(m + 1) * 128, i0:i0 + IC2],
                                      ob[:, :])

    nc.compile()
    return nc


def make_in_maps(cfg, x, cos, sin, Wq, Wk, Wv, Wo):
    """Host-side prep: transpose/slice full inputs into per-core input maps."""
    B, S, E = cfg["B"], cfg["S"], cfg["E"]
    NH, NKV, HD, NCORES = cfg["NH"], cfg["NKV"], cfg["HD"], cfg["ncores"]
    HPC = NH // NCORES
    QH = HPC * HD
    KVPC = NKV // NCORES

    x = np.asarray(x, dtype=np.float32)
    cos = np.asarray(cos, dtype=np.float32)
    sin = np.asarray(sin, dtype=np.float32)
    Wq = np.asarray(Wq, dtype=np.float32)
    Wk = np.asarray(Wk, dtype=np.float32)
    Wv = np.asarray(Wv, dtype=np.float32)
    Wo = np.asarray(Wo, dtype=np.float32)

    if cfg.get("mmdt", "bf16") == "bf16":
        import ml_dtypes
        mmnp = ml_dtypes.bfloat16
    else:
        mmnp = np.float32
    xT = np.ascontiguousarray(x.reshape(B * S, E).T.astype(mmnp))
    cos_t = cos.T[:HD]                        # [64, S]
    cosT = np.ascontiguousarray(np.concatenate([cos_t, cos_t], axis=0))
    sin_t = sin.T[:HD].copy()
    sin_t[:HD // 2] *= -1.0                   # signed sin for rotate-half
    sinT = np.ascontiguousarray(np.concatenate([sin_t, sin_t], axis=0))

    in_maps = []
    for c in range(NCORES):
        qsl = slice(c * QH, (c + 1) * QH)
        ksl = slice(c * KVPC * HD, (c + 1) * KVPC * HD)
        wq = np.ascontiguousarray(Wq[qsl, :].T.astype(mmnp))
        wkv = np.ascontiguousarray(
            np.concatenate([Wk[ksl, :].T, Wv[ksl, :].T], axis=1).astype(mmnp))
        wo = np.ascontiguousarray(Wo[qsl, :].T.astype(mmnp))
        in_maps.append(dict(xT=xT, wqT=wq, wkvT=wkv, woT=wo,
                            cosT=cosT, sinT=sinT))
    return in_maps


def assemble_output(cfg, results):
    B, S, E = cfg["B"], cfg["S"], cfg["E"]
    outT = np.concatenate([r["outT"] for r in results], axis=0)  # [E, B*S]
    return np.ascontiguousarray(outT.T.reshape(B, S, E).astype(np.float32))


def kernel(x, mask, cos, sin, Wq, Wk, Wv, Wo):
    global LAST_RESULTS
    _ensure_concourse()
    from concourse import bass_utils

    cfg = FULL_CFG
    nc = build_gqa(cfg)
    in_maps = make_in_maps(cfg, x, cos, sin, Wq, Wk, Wv, Wo)
    res = bass_utils.run_bass_kernel_spmd(
        nc, in_maps, core_ids=list(range(cfg["ncores"])))
    LAST_RESULTS = res
    return assemble_output(cfg, res.results)



# revision 15
# speedup vs baseline: 1.2633x; 1.2633x over previous
"""GroupedQueryAttention TRN2 Bass kernel.

Strategy (8 NeuronCores, tensor-parallel over heads):
  - Each core owns 4 q-heads (one kv head, GQA group of 4).
  - Host pre-transposes x and the weight slices so every matmul operand
    already has its contraction dim on SBUF partitions.
  - Pipelined schedule to keep TensorE busy and the HAM clock warm:
      QKV(b0) -> [attn(b0,qt) + interleaved QKV(b1) chunks]
      -> AllGather(b0) || [attn(b1,qt) + interleaved out-proj(b0) chunks]
      -> AllGather(b1) -> out-proj(b1)
  - Attention processes head PAIRS so the two 64-contraction score matmuls
    land in different PE row groups and run concurrently.
  - Softmax denominator comes from a ones-column in the V operand; the
    reciprocal is batched per q-block (4 heads at once, approx-fast DVE op).
  - RoPE rotate-half copies run on ScalarE (idle outside attention), the
    multiplies on VectorE in bf16.
"""

import os
import sys

import numpy as np


def _ensure_concourse():
    try:
        import concourse.bass  # noqa: F401
    except ImportError:
        for p in ("/opt/trn_rl_repo", "/root/.axon_site/_ro/trn_rl_repo"):
            if os.path.isdir(p) and p not in sys.path:
                sys.path.insert(0, p)
        import concourse.bass  # noqa: F401


FULL_CFG = dict(B=2, S=2048, E=2048, NH=32, NKV=8, HD=64, ncores=8, IC=512, IC2=512)

LAST_RESULTS = None  # BassKernelResults of the most recent kernel() call


def build_gqa(cfg):
    """Build the Bass module for one core's SPMD program. Returns nc."""
    _ensure_concourse()
    from contextlib import ExitStack

    import concourse.mybir as mybir
    import concourse.tile as tile
    from concourse import bacc
    from concourse.masks import make_identity

    dt = mybir.dt
    f32 = dt.float32
    bf16 = dt.bfloat16
    Exp = mybir.ActivationFunctionType.Exp

    B, S, E = cfg["B"], cfg["S"], cfg["E"]
    NH, NKV, HD = cfg["NH"], cfg["NKV"], cfg["HD"]
    NCORES = cfg["ncores"]
    HPC = NH // NCORES          # q heads per core
    assert HPC == 4 and HD == 64
    QH = HPC * HD               # 256: per-core q/ctx/out rows
    KVD = 2 * HD                # 128: packed K|V projection width
    NI = B * S                  # total tokens
    ET = E // 128               # contraction tiles
    IC = cfg["IC"]              # QKV token chunk (512)
    IC2 = cfg["IC2"]            # out-proj token chunk (512)
    QB = 512                    # attention q block
    KB = 128                    # attention k block
    NQT = S // QB               # q blocks per batch
    SKT = S // KB               # k tiles per batch
    NKTILES = NI // KB          # total k tiles (both batches)
    CPB = S // IC               # QKV chunks per batch
    P5C = S // IC2              # out-proj chunks per batch
    scale = 1.0 / float(np.sqrt(HD))

    nc = bacc.Bacc("TRN2", target_bir_lowering=False, debug=False,
                   num_devices=NCORES)

    xT = nc.dram_tensor("xT", [E, NI], bf16, kind="ExternalInput").ap()
    wqT = nc.dram_tensor("wqT", [E, QH], bf16, kind="ExternalInput").ap()
    wkvT = nc.dram_tensor("wkvT", [E, KVD], bf16, kind="ExternalInput").ap()
    woT = nc.dram_tensor("woT", [E, QH], bf16, kind="ExternalInput").ap()
    cosT = nc.dram_tensor("cosT", [128, S], bf16, kind="ExternalInput").ap()
    sinT = nc.dram_tensor("sinT", [128, S], bf16, kind="ExternalInput").ap()
    outT = nc.dram_tensor("outT", [QH, NI], f32, kind="ExternalOutput").ap()

    with tile.TileContext(nc) as tc, ExitStack() as persist:
        const = persist.enter_context(tc.tile_pool(name="const", bufs=1))
        qt_pool = persist.enter_context(tc.tile_pool(name="qt", bufs=1))
        kt_pool = persist.enter_context(tc.tile_pool(name="kt", bufs=1))
        vaug_pool = persist.enter_context(tc.tile_pool(name="vaug", bufs=1))
        ctxsb_pool = persist.enter_context(tc.tile_pool(name="ctxsb", bufs=1))
        stream_pool = persist.enter_context(tc.tile_pool(name="stream", bufs=2))
        rope_pool = persist.enter_context(tc.tile_pool(name="rope", bufs=2))
        vs_pool = persist.enter_context(tc.tile_pool(name="vs", bufs=2))
        exp_pool = persist.enter_context(tc.tile_pool(name="exp", bufs=3))
        rb_pool = persist.enter_context(tc.tile_pool(name="rb", bufs=2))
        cu_pool = persist.enter_context(tc.tile_pool(name="cu", bufs=3))
        ob_pool = persist.enter_context(tc.tile_pool(name="ob", bufs=2))
        proj_ps = persist.enter_context(
            tc.tile_pool(name="proj_ps", bufs=2, space="PSUM"))
        scores_ps = persist.enter_context(
            tc.tile_pool(name="scores_ps", bufs=2, space="PSUM"))
        ctx_ps_pool = persist.enter_context(
            tc.tile_pool(name="ctx_ps", bufs=2, space="PSUM"))
        dram = persist.enter_context(
            tc.tile_pool(name="dram", bufs=1, space="DRAM"))

        # ---- constants / weights (prefetched up front)
        ident = const.tile([128, 128], bf16, name="ident", tag="ident")
        make_identity(nc, ident[:, :])
        cos_sb = const.tile([128, S], bf16, name="cos_sb", tag="cos")
        nc.sync.dma_start(cos_sb[:, :], cosT)
        sin_sb = const.tile([128, S], bf16, name="sin_sb", tag="sin")
        nc.sync.dma_start(sin_sb[:, :], sinT)
        wq_sb = const.tile([128, ET, QH], bf16, name="wq_sb", tag="wq")
        nc.sync.dma_start(wq_sb[:, :, :],
                          wqT.rearrange("(t p) o -> p t o", p=128))
        wkv_sb = const.tile([128, ET, KVD], bf16, name="wkv_sb", tag="wkv")
        nc.sync.dma_start(wkv_sb[:, :, :],
                          wkvT.rearrange("(t p) o -> p t o", p=128))
        wo_sb = const.tile([128, ET, QH], bf16, name="wo_sb", tag="wo")
        nc.sync.dma_start(wo_sb[:, :, :],
                          woT.rearrange("(t p) o -> p t o", p=128))
        nqb = QB // KB
        mask_sb = const.tile([128, nqb, 2 * QB], bf16, name="mask_sb", tag="mask")
        nc.gpsimd.memset(mask_sb[:, :, :], 1.0)
        for j in range(nqb):
            for hf in range(2):
                # keep where dq >= dk + KB*j (causal within diagonal band)
                nc.gpsimd.affine_select(
                    out=mask_sb[:, j, hf * QB:(hf + 1) * QB],
                    in_=mask_sb[:, j, hf * QB:(hf + 1) * QB],
                    pattern=[[1, QB]], compare_op=mybir.AluOpType.is_ge,
                    fill=0.0, base=-KB * j, channel_multiplier=-1)

        # persistent activations
        qt_sb = [qt_pool.tile([128, NI], bf16, name=f"qt{m}", tag=f"qt{m}")
                 for m in range(HPC // 2)]
        kt_sb = kt_pool.tile([128, NI], bf16, tag="ktd")  # K^T duplicated 2x
        vaug = [vaug_pool.tile([128, HD + 1], bf16, name=f"va{k}", tag=f"va{k}")
                for k in range(NKTILES)]
        for k in range(NKTILES):
            nc.gpsimd.memset(vaug[k][:, HD:HD + 1], 1.0)
        ctx_sb = [ctxsb_pool.tile([128, NI], bf16, name=f"cx{m}", tag=f"cx{m}")
                  for m in range(HPC // 2)]

        # internal DRAM for per-batch collectives
        cc_in = [dram.tile([QH, S], bf16, name=f"cc_in{b}", tag=f"ccin{b}")
                 for b in range(B)]
        cc_out = [dram.tile([E, S], bf16, name=f"cc_out{b}", tag=f"ccout{b}",
                            addr_space="Shared")
                  for b in range(B)]

        def rope(dst, src_ps, parts, s0, ln):
            # dst = src*cos + rot_half(src)*signed_sin ; src PSUM f32, dst bf16
            # ScalarE does the (same-partition) PSUM->SBUF cast; VectorE does
            # the +-32-partition rotate-half copies and the bf16 muls/add.
            psb = rope_pool.tile([128, IC], bf16, name="psb", tag="psb")
            qs_t = rope_pool.tile([128, IC], bf16, name="qs_t", tag="qs")
            t1_t = rope_pool.tile([128, IC], bf16, name="t1_t", tag="t1")
            nc.scalar.copy(psb[:parts, :ln], src_ps[:parts, :ln])
            for h0 in range(0, parts, 64):
                nc.vector.tensor_copy(qs_t[h0:h0 + 32, :ln],
                                      psb[h0 + 32:h0 + 64, :ln])
                nc.vector.tensor_copy(qs_t[h0 + 32:h0 + 64, :ln],
                                      psb[h0:h0 + 32, :ln])
            nc.vector.tensor_mul(t1_t[:parts, :ln], psb[:parts, :ln],
                                 cos_sb[:parts, s0:s0 + ln])
            nc.vector.tensor_mul(qs_t[:parts, :ln], qs_t[:parts, :ln],
                                 sin_sb[:parts, s0:s0 + ln])
            nc.vector.tensor_add(dst, t1_t[:parts, :ln], qs_t[:parts, :ln])

        def qkv_chunk(gch):
            """QKV projection + RoPE + V transpose for 512 tokens.
            gch: global chunk id (0..NI/IC-1)."""
            i0 = gch * IC
            s0 = i0 % S
            xt = stream_pool.tile([128, ET, IC], bf16, name="xt", tag="xt")
            nc.sync.dma_start(
                xt[:, :, :],
                xT[:, i0:i0 + IC].rearrange("(t p) i -> p t i", p=128))
            for m in range(HPC // 2):
                q_ps = proj_ps.tile([128, IC], f32, name="pps", tag="proj")
                for t in range(ET):
                    nc.tensor.matmul(
                        q_ps[:, :],
                        wq_sb[:, t, m * 128:(m + 1) * 128],
                        xt[:, t, :],
                        start=(t == 0), stop=(t == ET - 1))
                rope(qt_sb[m][:, i0:i0 + IC], q_ps, 128, s0, IC)
            kv_ps = proj_ps.tile([128, IC], f32, name="pps", tag="proj")
            for t in range(ET):
                nc.tensor.matmul(
                    kv_ps[:, :],
                    wkv_sb[:, t, :],
                    xt[:, t, :],
                    start=(t == 0), stop=(t == ET - 1))
            rope(kt_sb[0:64, i0:i0 + IC], kv_ps, 64, s0, IC)
            nc.vector.tensor_copy(kt_sb[64:128, i0:i0 + IC],
                                  kt_sb[0:64, i0:i0 + IC])
            vs = vs_pool.tile([128, IC], bf16, name="vs", tag="vs")
            nc.scalar.copy(vs[64:128, :], kv_ps[64:128, :])
            for j in range(IC // 128):
                kidx = (i0 + j * 128) // 128
                vt_ps = proj_ps.tile([128, IC], bf16, name="pps", tag="proj")
                nc.tensor.transpose(vt_ps[:, 0:64],
                                    vs[64:128, j * 128:(j + 1) * 128],
                                    ident[64:128, 64:128])
                nc.scalar.copy(vaug[kidx][:, 0:HD], vt_ps[:, 0:64])

        def attn_block(b, qt, mt):
            """Attention for head pair (2mt, 2mt+1) of q-block qt of batch b.
            The two 64-contraction score matmuls use different PE row groups
            and run concurrently."""
            tok0 = b * S + qt * QB
            qA = qt_sb[mt][0:64, tok0:tok0 + QB]
            qB = qt_sb[mt][64:128, tok0:tok0 + QB]
            ctxA = ctx_ps_pool.tile([128, QB], f32, name="ctxA", tag="ctx")
            ctxB = ctx_ps_pool.tile([128, QB], f32, name="ctxB", tag="ctx")
            nkt = (qt + 1) * (QB // KB)
            for kt in range(nkt):
                ks = b * S + kt * KB
                s_ps = scores_ps.tile([128, 2 * QB], f32, name="s_ps", tag="s")
                nc.tensor.matmul(s_ps[:, 0:QB],
                                 kt_sb[0:64, ks:ks + KB], qA,
                                 start=True, stop=True)
                nc.tensor.matmul(s_ps[:, QB:2 * QB],
                                 kt_sb[64:128, ks:ks + KB], qB,
                                 start=True, stop=True)
                e_t = exp_pool.tile([128, 2 * QB], bf16, name="e_t", tag="e")
                nc.scalar.activation(e_t[:, :], s_ps[:, :], Exp, scale=scale)
                j = kt - qt * (QB // KB)
                if j >= 0:  # diagonal block: causal mask (both heads at once)
                    nc.vector.tensor_mul(e_t[:, :], e_t[:, :], mask_sb[:, j, :])
                nc.tensor.matmul(ctxA[0:HD + 1, :], vaug[b * SKT + kt][:, :],
                                 e_t[:, 0:QB],
                                 start=(kt == 0), stop=(kt == nkt - 1))
                nc.tensor.matmul(ctxB[0:HD + 1, :], vaug[b * SKT + kt][:, :],
                                 e_t[:, QB:2 * QB],
                                 start=(kt == 0), stop=(kt == nkt - 1))
            for a, ctxX in ((0, ctxA), (1, ctxB)):
                # one copy frees the PSUM bank; den row then staged to
                # partition 0 (recip_approx_fast and partition_broadcast
                # only work with base-0 APs)
                ctxu = cu_pool.tile([128, QB], bf16, name="ctxu", tag="cu")
                nc.vector.tensor_copy(ctxu[0:HD + 1, :], ctxX[0:HD + 1, :])
                den1 = rb_pool.tile([1, QB], f32, name="den1", tag="den1")
                nc.vector.tensor_copy(den1[0:1, :], ctxu[HD:HD + 1, :])
                recip = rb_pool.tile([1, QB], f32, name="recip", tag="recip")
                nc.vector.reciprocal_approx_fast(recip[0:1, :], den1[0:1, :])
                rbc = rb_pool.tile([64, QB], f32, name="rbc", tag="rb")
                nc.gpsimd.partition_broadcast(rbc[:, :], recip[0:1, :])
                nc.vector.tensor_mul(
                    ctx_sb[mt][a * 64:a * 64 + 64, tok0:tok0 + QB],
                    ctxu[0:64, :], rbc[:, :])

        def attn_qblock(b, qt):
            for mt in range(HPC // 2):
                attn_block(b, qt, mt)

        def gather_batch(b):
            for m in range(HPC // 2):
                nc.sync.dma_start(cc_in[b][m * 128:(m + 1) * 128, :],
                                  ctx_sb[m][:, b * S:b * S + S])
            nc.gpsimd.collective_compute(
                "AllGather", mybir.AluOpType.bypass,
                replica_groups=[list(range(NCORES))],
                ins=[cc_in[b][:, :]],
                outs=[cc_out[b][:, :]])

        def p5_chunk(b, ch):
            """Output projection for 512 tokens of batch b."""
            t0 = ch * IC2
            ct = stream_pool.tile([128, ET, IC2], bf16, name="ct", tag="xt")
            nc.sync.dma_start(
                ct[:, :, :],
                cc_out[b][:, t0:t0 + IC2].rearrange("(t p) i -> p t i", p=128))
            for m in range(HPC // 2):
                o_ps = proj_ps.tile([128, IC2], f32, name="ops", tag="proj")
                for t in range(ET):
                    nc.tensor.matmul(
                        o_ps[:, :],
                        wo_sb[:, t, m * 128:(m + 1) * 128],
                        ct[:, t, :],
                        start=(t == 0), stop=(t == ET - 1))
                ob = ob_pool.tile([128, IC2], f32, name="ob", tag="ob")
                nc.vector.tensor_copy(ob[:, :], o_ps[:, :])
                nc.sync.dma_start(
                    outT[m * 128:(m + 1) * 128, b * S + t0:b * S + t0 + IC2],
                    ob[:, :])

        # ================= schedule =================
        # batch-0 QKV
        for ch in range(CPB):
            qkv_chunk(ch)
        # batch-0 attention with batch-1 QKV chunks filling PE idle time
        qkv1_sched = [[], [0], [1, 2], [3]]
        for qt in range(NQT):
            attn_qblock(0, qt)
            for ch in qkv1_sched[qt]:
                qkv_chunk(CPB + ch)
        gather_batch(0)
        # batch-1 attention with batch-0 out-proj chunks filling PE idle time
        for qt in range(NQT):
            attn_qblock(1, qt)
            p5_chunk(0, qt)
        gather_batch(1)
        for ch in range(P5C):
            p5_chunk(1, ch)

    nc.compile()
    return nc


def make_in_maps(cfg, x, cos, sin, Wq, Wk, Wv, Wo):
    """Host-side prep: transpose/slice full inputs into per-core input maps."""
    B, S, E = cfg["B"], cfg["S"], cfg["E"]
    NH, NKV, HD, NCORES = cfg["NH"], cfg["NKV"], cfg["HD"], cfg["ncores"]
    HPC = NH // NCORES
    QH = HPC * HD
    KVPC = NKV // NCORES

    import ml_dtypes
    mmnp = ml_dtypes.bfloat16

    x = np.asarray(x, dtype=np.float32)
    cos = np.asarray(cos, dtype=np.float32)
    sin = np.asarray(sin, dtype=np.float32)
    Wq = np.asarray(Wq, dtype=np.float32)
    Wk = np.asarray(Wk, dtype=np.float32)
    Wv = np.asarray(Wv, dtype=np.float32)
    Wo = np.asarray(Wo, dtype=np.float32)

    xT = np.ascontiguousarray(x.reshape(B * S, E).T.astype(mmnp))
    cos_t = cos.T[:HD]                        # [64, S]
    cosT = np.ascontiguousarray(
        np.concatenate([cos_t, cos_t], axis=0).astype(mmnp))
    sin_t = sin.T[:HD].copy()
    sin_t[:HD // 2] *= -1.0                   # signed sin for rotate-half
    sinT = np.ascontiguousarray(
        np.concatenate([sin_t, sin_t], axis=0).astype(mmnp))

    in_maps = []
    for c in range(NCORES):
        qsl = slice(c * QH, (c + 1) * QH)
        ksl = slice(c * KVPC * HD, (c + 1) * KVPC * HD)
        wq = np.ascontiguousarray(Wq[qsl, :].T.astype(mmnp))
        wkv = np.ascontiguousarray(
            np.concatenate([Wk[ksl, :].T, Wv[ksl, :].T], axis=1).astype(mmnp))
        wo = np.ascontiguousarray(Wo[qsl, :].T.astype(mmnp))
        in_maps.append(dict(xT=xT, wqT=wq, wkvT=wkv, woT=wo,
                            cosT=cosT, sinT=sinT))
    return in_maps


def assemble_output(cfg, results):
    B, S, E = cfg["B"], cfg["S"], cfg["E"]
    outT = np.concatenate([r["outT"] for r in results], axis=0)  # [E, B*S]
    return np.ascontiguousarray(outT.T.reshape(B, S, E).astype(np.float32))


def kernel(x, mask, cos, sin, Wq, Wk, Wv, Wo):
    global LAST_RESULTS
    _ensure_concourse()
    from concourse import bass_utils

    cfg = FULL_CFG
    nc = build_gqa(cfg)
    in_maps = make_in_maps(cfg, x, cos, sin, Wq, Wk, Wv, Wo)
    res = bass_utils.run_bass_kernel_spmd(
        nc, in_maps, core_ids=list(range(cfg["ncores"])))
    LAST_RESULTS = res
    return assemble_output(cfg, res.results)


# revision 21
# speedup vs baseline: 1.4924x; 1.1814x over previous
"""GroupedQueryAttention TRN2 Bass kernel.

Strategy (8 NeuronCores, tensor-parallel over heads):
  - Each core owns 4 q-heads (one kv head, GQA group of 4).
  - Host pre-transposes x and the weight slices so every matmul operand
    already has its contraction dim on SBUF partitions.
  - Pipelined schedule to keep TensorE busy and the HAM clock warm:
      QKV(b0) -> [attn(b0,qt) + interleaved QKV(b1) chunks]
      -> AllGather(b0) || [attn(b1,qt) + interleaved out-proj(b0) chunks]
      -> AllGather(b1) -> out-proj(b1)
  - Attention processes head PAIRS so the two 64-contraction score matmuls
    land in different PE row groups and run concurrently.
  - Softmax denominator comes from a ones-column in the V operand; the
    reciprocal is batched per q-block (4 heads at once, approx-fast DVE op).
  - RoPE rotate-half copies run on ScalarE (idle outside attention), the
    multiplies on VectorE in bf16.
"""

import os
import sys

import numpy as np


def _ensure_concourse():
    try:
        import concourse.bass  # noqa: F401
    except ImportError:
        for p in ("/opt/trn_rl_repo", "/root/.axon_site/_ro/trn_rl_repo"):
            if os.path.isdir(p) and p not in sys.path:
                sys.path.insert(0, p)
        import concourse.bass  # noqa: F401


FULL_CFG = dict(B=2, S=2048, E=2048, NH=32, NKV=8, HD=64, ncores=8, IC=512, IC2=512)

LAST_RESULTS = None  # BassKernelResults of the most recent kernel() call


def build_gqa(cfg):
    """Build the Bass module for one core's SPMD program. Returns nc."""
    _ensure_concourse()
    from contextlib import ExitStack

    import concourse.mybir as mybir
    import concourse.tile as tile
    from concourse import bacc
    from concourse.masks import make_identity

    dt = mybir.dt
    f32 = dt.float32
    bf16 = dt.bfloat16
    Exp = mybir.ActivationFunctionType.Exp

    B, S, E = cfg["B"], cfg["S"], cfg["E"]
    NH, NKV, HD = cfg["NH"], cfg["NKV"], cfg["HD"]
    NCORES = cfg["ncores"]
    HPC = NH // NCORES          # q heads per core
    assert HPC == 4 and HD == 64
    QH = HPC * HD               # 256: per-core q/ctx/out rows
    KVD = 2 * HD                # 128: packed K|V projection width
    NI = B * S                  # total tokens
    ET = E // 128               # contraction tiles
    IC = cfg["IC"]              # QKV token chunk (512)
    IC2 = cfg["IC2"]            # out-proj token chunk (512)
    QB = 512                    # attention q block
    KB = 128                    # attention k block
    NQT = S // QB               # q blocks per batch
    SKT = S // KB               # k tiles per batch
    NKTILES = NI // KB          # total k tiles (both batches)
    CPB = S // IC               # QKV chunks per batch
    P5C = S // IC2              # out-proj chunks per batch
    scale = 1.0 / float(np.sqrt(HD))

    nc = bacc.Bacc("TRN2", target_bir_lowering=False, debug=False,
                   num_devices=NCORES)

    xT = nc.dram_tensor("xT", [E, NI], bf16, kind="ExternalInput").ap()
    wqT = nc.dram_tensor("wqT", [E, QH], bf16, kind="ExternalInput").ap()
    wkvT = nc.dram_tensor("wkvT", [E, KVD], bf16, kind="ExternalInput").ap()
    woT = nc.dram_tensor("woT", [E, QH], bf16, kind="ExternalInput").ap()
    cosT = nc.dram_tensor("cosT", [128, S], bf16, kind="ExternalInput").ap()
    sinT = nc.dram_tensor("sinT", [128, S], bf16, kind="ExternalInput").ap()
    outT = nc.dram_tensor("outT", [QH, NI], f32, kind="ExternalOutput").ap()

    with tile.TileContext(nc) as tc, ExitStack() as persist:
        const = persist.enter_context(tc.tile_pool(name="const", bufs=1))
        qt_pool = persist.enter_context(tc.tile_pool(name="qt", bufs=1))
        kt_pool = persist.enter_context(tc.tile_pool(name="kt", bufs=1))
        vaug_pool = persist.enter_context(tc.tile_pool(name="vaug", bufs=1))
        ctxsb_pool = persist.enter_context(tc.tile_pool(name="ctxsb", bufs=1))
        stream_pool = persist.enter_context(tc.tile_pool(name="stream", bufs=2))
        rope_pool = persist.enter_context(tc.tile_pool(name="rope", bufs=2))
        vs_pool = persist.enter_context(tc.tile_pool(name="vs", bufs=2))
        exp_pool = persist.enter_context(tc.tile_pool(name="exp", bufs=3))
        rb_pool = persist.enter_context(tc.tile_pool(name="rb", bufs=2))
        cu_pool = persist.enter_context(tc.tile_pool(name="cu", bufs=3))
        ob_pool = persist.enter_context(tc.tile_pool(name="ob", bufs=2))
        proj_ps = persist.enter_context(
            tc.tile_pool(name="proj_ps", bufs=2, space="PSUM"))
        scores_ps = persist.enter_context(
            tc.tile_pool(name="scores_ps", bufs=2, space="PSUM"))
        ctx_ps_pool = persist.enter_context(
            tc.tile_pool(name="ctx_ps", bufs=2, space="PSUM"))
        dram = persist.enter_context(
            tc.tile_pool(name="dram", bufs=1, space="DRAM"))

        # ---- constants / weights (prefetched up front)
        ident = const.tile([128, 128], bf16, name="ident", tag="ident")
        make_identity(nc, ident[:, :])
        cos_sb = const.tile([128, S], bf16, name="cos_sb", tag="cos")
        nc.sync.dma_start(cos_sb[:, :], cosT)
        sin_sb = const.tile([128, S], bf16, name="sin_sb", tag="sin")
        nc.sync.dma_start(sin_sb[:, :], sinT)
        wq_sb = const.tile([128, ET, QH], bf16, name="wq_sb", tag="wq")
        nc.sync.dma_start(wq_sb[:, :, :],
                          wqT.rearrange("(t p) o -> p t o", p=128))
        wkv_sb = const.tile([128, ET, KVD], bf16, name="wkv_sb", tag="wkv")
        nc.sync.dma_start(wkv_sb[:, :, :],
                          wkvT.rearrange("(t p) o -> p t o", p=128))
        wo_sb = const.tile([128, ET, QH], bf16, name="wo_sb", tag="wo")
        nc.sync.dma_start(wo_sb[:, :, :],
                          woT.rearrange("(t p) o -> p t o", p=128))
        # triangle mask (keep where dq >= dk), duplicated for the head pair;
        # diagonal k-tiles are trimmed to their live columns so one mask fits all
        mask_sb = const.tile([128, 2, QB], bf16, name="mask_sb", tag="mask")
        nc.gpsimd.memset(mask_sb[:, :, :], 1.0)
        for hf in range(2):
            nc.gpsimd.affine_select(
                out=mask_sb[:, hf, :], in_=mask_sb[:, hf, :],
                pattern=[[1, QB]], compare_op=mybir.AluOpType.is_ge,
                fill=0.0, base=0, channel_multiplier=-1)

        # persistent activations
        qt_sb = [qt_pool.tile([128, NI], bf16, name=f"qt{m}", tag=f"qt{m}")
                 for m in range(HPC // 2)]
        kt_sb = kt_pool.tile([128, NI], bf16, tag="ktd")  # K^T duplicated 2x
        vaug = [vaug_pool.tile([128, HD + 1], bf16, name=f"va{k}", tag=f"va{k}")
                for k in range(NKTILES)]
        for k in range(NKTILES):
            nc.gpsimd.memset(vaug[k][:, HD:HD + 1], 1.0)
        ctx_sb = [ctxsb_pool.tile([128, NI], bf16, name=f"cx{m}", tag=f"cx{m}")
                  for m in range(HPC // 2)]

        # internal DRAM for half-batch collectives (issued mid-attention so
        # the out-projection never waits on a full batch's gather)
        HS = S // 2
        cc_in = [dram.tile([QH, HS], bf16, name=f"cc_in{h}", tag=f"ccin{h}")
                 for h in range(2 * B)]
        cc_out = [dram.tile([E, HS], bf16, name=f"cc_out{h}", tag=f"ccout{h}",
                            addr_space="Shared")
                  for h in range(2 * B)]

        def rope(dst, src_ps, parts, s0, ln):
            # dst = src*cos + rot_half(src)*signed_sin ; src PSUM f32, dst bf16
            # ScalarE does the (same-partition) PSUM->SBUF cast; VectorE does
            # the +-32-partition rotate-half copies and the bf16 muls/add.
            psb = rope_pool.tile([128, IC], bf16, name="psb", tag="psb")
            qs_t = rope_pool.tile([128, IC], bf16, name="qs_t", tag="qs")
            t1_t = rope_pool.tile([128, IC], bf16, name="t1_t", tag="t1")
            nc.scalar.copy(psb[:parts, :ln], src_ps[:parts, :ln])
            for h0 in range(0, parts, 64):
                nc.vector.tensor_copy(qs_t[h0:h0 + 32, :ln],
                                      psb[h0 + 32:h0 + 64, :ln])
                nc.vector.tensor_copy(qs_t[h0 + 32:h0 + 64, :ln],
                                      psb[h0:h0 + 32, :ln])
            nc.vector.tensor_mul(t1_t[:parts, :ln], psb[:parts, :ln],
                                 cos_sb[:parts, s0:s0 + ln])
            nc.vector.tensor_mul(qs_t[:parts, :ln], qs_t[:parts, :ln],
                                 sin_sb[:parts, s0:s0 + ln])
            nc.vector.tensor_add(dst, t1_t[:parts, :ln], qs_t[:parts, :ln])

        def qkv_chunk(gch):
            """QKV projection + RoPE + V transpose for 512 tokens.
            gch: global chunk id (0..NI/IC-1)."""
            i0 = gch * IC
            s0 = i0 % S
            xt = stream_pool.tile([128, ET, IC], bf16, name="xt", tag="xt")
            nc.sync.dma_start(
                xt[:, :, :],
                xT[:, i0:i0 + IC].rearrange("(t p) i -> p t i", p=128))
            for m in range(HPC // 2):
                q_ps = proj_ps.tile([128, IC], f32, name="pps", tag="proj")
                for t in range(ET):
                    nc.tensor.matmul(
                        q_ps[:, :],
                        wq_sb[:, t, m * 128:(m + 1) * 128],
                        xt[:, t, :],
                        start=(t == 0), stop=(t == ET - 1))
                rope(qt_sb[m][:, i0:i0 + IC], q_ps, 128, s0, IC)
            kv_ps = proj_ps.tile([128, IC], f32, name="pps", tag="proj")
            for t in range(ET):
                nc.tensor.matmul(
                    kv_ps[:, :],
                    wkv_sb[:, t, :],
                    xt[:, t, :],
                    start=(t == 0), stop=(t == ET - 1))
            rope(kt_sb[0:64, i0:i0 + IC], kv_ps, 64, s0, IC)
            nc.vector.tensor_copy(kt_sb[64:128, i0:i0 + IC],
                                  kt_sb[0:64, i0:i0 + IC])
            vs = vs_pool.tile([128, IC], bf16, name="vs", tag="vs")
            nc.scalar.copy(vs[64:128, :], kv_ps[64:128, :])
            for j in range(IC // 128):
                kidx = (i0 + j * 128) // 128
                vt_ps = proj_ps.tile([128, IC], bf16, name="pps", tag="proj")
                nc.tensor.transpose(vt_ps[:, 0:64],
                                    vs[64:128, j * 128:(j + 1) * 128],
                                    ident[64:128, 64:128])
                nc.scalar.copy(vaug[kidx][:, 0:HD], vt_ps[:, 0:64])

        def attn_block(b, qt, mt):
            """Attention for head pair (2mt, 2mt+1) of q-block qt of batch b.
            The two 64-contraction score matmuls use different PE row groups
            and run concurrently."""
            tok0 = b * S + qt * QB
            ctxA = ctx_ps_pool.tile([128, QB], f32, name="ctxA", tag="ctx")
            ctxB = ctx_ps_pool.tile([128, QB], f32, name="ctxB", tag="ctx")
            nkt = (qt + 1) * (QB // KB)
            for kt in range(nkt):
                ks = b * S + kt * KB
                j = kt - qt * (QB // KB)
                # diagonal k-tiles only touch q columns >= 128*j
                w0 = 128 * j if j > 0 else 0
                s_ps = scores_ps.tile([128, 2, QB], f32, name="s_ps", tag="s")
                nc.tensor.matmul(s_ps[:, 0, w0:QB],
                                 kt_sb[0:64, ks:ks + KB],
                                 qt_sb[mt][0:64, tok0 + w0:tok0 + QB],
                                 start=True, stop=True)
                nc.tensor.matmul(s_ps[:, 1, w0:QB],
                                 kt_sb[64:128, ks:ks + KB],
                                 qt_sb[mt][64:128, tok0 + w0:tok0 + QB],
                                 start=True, stop=True)
                e_t = exp_pool.tile([128, 2, QB], bf16, name="e_t", tag="e")
                nc.scalar.activation(e_t[:, :, w0:QB], s_ps[:, :, w0:QB],
                                     Exp, scale=scale)
                if j >= 0:  # diagonal block: causal mask (both heads at once)
                    nc.vector.tensor_mul(e_t[:, :, w0:QB], e_t[:, :, w0:QB],
                                         mask_sb[:, :, 0:QB - w0])
                nc.tensor.matmul(ctxA[0:HD + 1, w0:QB], vaug[b * SKT + kt][:, :],
                                 e_t[:, 0, w0:QB],
                                 start=(kt == 0), stop=(kt == nkt - 1))
                nc.tensor.matmul(ctxB[0:HD + 1, w0:QB], vaug[b * SKT + kt][:, :],
                                 e_t[:, 1, w0:QB],
                                 start=(kt == 0), stop=(kt == nkt - 1))
            for a, ctxX in ((0, ctxA), (1, ctxB)):
                # one copy frees the PSUM bank; den row then staged to
                # partition 0 (recip_approx_fast and partition_broadcast
                # only work with base-0 APs)
                ctxu = cu_pool.tile([128, QB], bf16, name="ctxu", tag="cu")
                nc.vector.tensor_copy(ctxu[0:HD + 1, :], ctxX[0:HD + 1, :])
                den1 = rb_pool.tile([1, QB], f32, name="den1", tag="den1")
                nc.vector.tensor_copy(den1[0:1, :], ctxu[HD:HD + 1, :])
                recip = rb_pool.tile([1, QB], f32, name="recip", tag="recip")
                nc.vector.reciprocal_approx_fast(recip[0:1, :], den1[0:1, :])
                rbc = rb_pool.tile([64, QB], f32, name="rbc", tag="rb")
                nc.gpsimd.partition_broadcast(rbc[:, :], recip[0:1, :])
                nc.vector.tensor_mul(
                    ctx_sb[mt][a * 64:a * 64 + 64, tok0:tok0 + QB],
                    ctxu[0:64, :], rbc[:, :])

        def attn_qblock(b, qt):
            for mt in range(HPC // 2):
                attn_block(b, qt, mt)

        def gather_half(b, half):
            h = 2 * b + half
            t0 = b * S + half * HS
            for m in range(HPC // 2):
                nc.sync.dma_start(cc_in[h][m * 128:(m + 1) * 128, :],
                                  ctx_sb[m][:, t0:t0 + HS])
            nc.gpsimd.collective_compute(
                "AllGather", mybir.AluOpType.bypass,
                replica_groups=[list(range(NCORES))],
                ins=[cc_in[h][:, :]],
                outs=[cc_out[h][:, :]])

        def p5_chunk(b, ch):
            """Output projection for 512 tokens of batch b."""
            t0 = ch * IC2
            h = 2 * b + (t0 // HS)
            lt = t0 % HS
            ct = stream_pool.tile([128, ET, IC2], bf16, name="ct", tag="xt")
            nc.sync.dma_start(
                ct[:, :, :],
                cc_out[h][:, lt:lt + IC2].rearrange("(t p) i -> p t i", p=128))
            for m in range(HPC // 2):
                o_ps = proj_ps.tile([128, IC2], f32, name="ops", tag="proj")
                for t in range(ET):
                    nc.tensor.matmul(
                        o_ps[:, :],
                        wo_sb[:, t, m * 128:(m + 1) * 128],
                        ct[:, t, :],
                        start=(t == 0), stop=(t == ET - 1))
                ob = ob_pool.tile([128, IC2], f32, name="ob", tag="ob")
                nc.vector.tensor_copy(ob[:, :], o_ps[:, :])
                nc.sync.dma_start(
                    outT[m * 128:(m + 1) * 128, b * S + t0:b * S + t0 + IC2],
                    ob[:, :])

        # ================= schedule =================
        # batch-0 QKV
        for ch in range(CPB):
            qkv_chunk(ch)
        # batch-0 attention; batch-1 QKV chunks fill PE idle time; the first
        # half-gather is issued as soon as q-blocks 0-1 are normalized
        attn_qblock(0, 0)
        attn_qblock(0, 1)
        qkv_chunk(CPB + 0)
        gather_half(0, 0)
        attn_qblock(0, 2)
        qkv_chunk(CPB + 1)
        qkv_chunk(CPB + 2)
        attn_qblock(0, 3)
        qkv_chunk(CPB + 3)
        gather_half(0, 1)
        # batch-1 attention; batch-0 out-proj chunks fill PE idle time
        attn_qblock(1, 0)
        p5_chunk(0, 0)
        attn_qblock(1, 1)
        p5_chunk(0, 1)
        gather_half(1, 0)
        attn_qblock(1, 2)
        p5_chunk(0, 2)
        attn_qblock(1, 3)
        p5_chunk(0, 3)
        gather_half(1, 1)
        for ch in range(P5C):
            p5_chunk(1, ch)

    nc.compile()
    return nc


def make_in_maps(cfg, x, cos, sin, Wq, Wk, Wv, Wo):
    """Host-side prep: transpose/slice full inputs into per-core input maps."""
    B, S, E = cfg["B"], cfg["S"], cfg["E"]
    NH, NKV, HD, NCORES = cfg["NH"], cfg["NKV"], cfg["HD"], cfg["ncores"]
    HPC = NH // NCORES
    QH = HPC * HD
    KVPC = NKV // NCORES

    import ml_dtypes
    mmnp = ml_dtypes.bfloat16

    x = np.asarray(x, dtype=np.float32)
    cos = np.asarray(cos, dtype=np.float32)
    sin = np.asarray(sin, dtype=np.float32)
    Wq = np.asarray(Wq, dtype=np.float32)
    Wk = np.asarray(Wk, dtype=np.float32)
    Wv = np.asarray(Wv, dtype=np.float32)
    Wo = np.asarray(Wo, dtype=np.float32)

    xT = np.ascontiguousarray(x.reshape(B * S, E).T.astype(mmnp))
    cos_t = cos.T[:HD]                        # [64, S]
    cosT = np.ascontiguousarray(
        np.concatenate([cos_t, cos_t], axis=0).astype(mmnp))
    sin_t = sin.T[:HD].copy()
    sin_t[:HD // 2] *= -1.0                   # signed sin for rotate-half
    sinT = np.ascontiguousarray(
        np.concatenate([sin_t, sin_t], axis=0).astype(mmnp))

    in_maps = []
    for c in range(NCORES):
        qsl = slice(c * QH, (c + 1) * QH)
        ksl = slice(c * KVPC * HD, (c + 1) * KVPC * HD)
        wq = np.ascontiguousarray(Wq[qsl, :].T.astype(mmnp))
        wkv = np.ascontiguousarray(
            np.concatenate([Wk[ksl, :].T, Wv[ksl, :].T], axis=1).astype(mmnp))
        wo = np.ascontiguousarray(Wo[qsl, :].T.astype(mmnp))
        in_maps.append(dict(xT=xT, wqT=wq, wkvT=wkv, woT=wo,
                            cosT=cosT, sinT=sinT))
    return in_maps


def assemble_output(cfg, results):
    B, S, E = cfg["B"], cfg["S"], cfg["E"]
    outT = np.concatenate([r["outT"] for r in results], axis=0)  # [E, B*S]
    return np.ascontiguousarray(outT.T.reshape(B, S, E).astype(np.float32))


def kernel(x, mask, cos, sin, Wq, Wk, Wv, Wo):
    global LAST_RESULTS
    _ensure_concourse()
    from concourse import bass_utils

    cfg = FULL_CFG
    nc = build_gqa(cfg)
    in_maps = make_in_maps(cfg, x, cos, sin, Wq, Wk, Wv, Wo)
    res = bass_utils.run_bass_kernel_spmd(
        nc, in_maps, core_ids=list(range(cfg["ncores"])))
    LAST_RESULTS = res
    return assemble_output(cfg, res.results)


# revision 24
# speedup vs baseline: 1.5054x; 1.0087x over previous
"""GroupedQueryAttention TRN2 Bass kernel.

Strategy (8 NeuronCores, tensor-parallel over heads):
  - Each core owns 4 q-heads (one kv head, GQA group of 4).
  - Host pre-transposes x and the weight slices so every matmul operand
    already has its contraction dim on SBUF partitions.
  - Pipelined schedule to keep TensorE busy and the HAM clock warm:
      QKV(b0) -> [attn(b0,qt) + interleaved QKV(b1) chunks]
      -> AllGather(b0) || [attn(b1,qt) + interleaved out-proj(b0) chunks]
      -> AllGather(b1) -> out-proj(b1)
  - Attention processes head PAIRS so the two 64-contraction score matmuls
    land in different PE row groups and run concurrently.
  - Softmax denominator comes from a ones-column in the V operand; the
    reciprocal is batched per q-block (4 heads at once, approx-fast DVE op).
  - RoPE rotate-half copies run on ScalarE (idle outside attention), the
    multiplies on VectorE in bf16.
"""

import os
import sys

import numpy as np


def _ensure_concourse():
    try:
        import concourse.bass  # noqa: F401
    except ImportError:
        for p in ("/opt/trn_rl_repo", "/root/.axon_site/_ro/trn_rl_repo"):
            if os.path.isdir(p) and p not in sys.path:
                sys.path.insert(0, p)
        import concourse.bass  # noqa: F401


FULL_CFG = dict(B=2, S=2048, E=2048, NH=32, NKV=8, HD=64, ncores=8, IC=512, IC2=512)

LAST_RESULTS = None  # BassKernelResults of the most recent kernel() call


def build_gqa(cfg):
    """Build the Bass module for one core's SPMD program. Returns nc."""
    _ensure_concourse()
    from contextlib import ExitStack

    import concourse.mybir as mybir
    import concourse.tile as tile
    from concourse import bacc
    from concourse.masks import make_identity

    dt = mybir.dt
    f32 = dt.float32
    bf16 = dt.bfloat16
    Exp = mybir.ActivationFunctionType.Exp

    B, S, E = cfg["B"], cfg["S"], cfg["E"]
    NH, NKV, HD = cfg["NH"], cfg["NKV"], cfg["HD"]
    NCORES = cfg["ncores"]
    HPC = NH // NCORES          # q heads per core
    assert HPC == 4 and HD == 64
    QH = HPC * HD               # 256: per-core q/ctx/out rows
    KVD = 2 * HD                # 128: packed K|V projection width
    NI = B * S                  # total tokens
    ET = E // 128               # contraction tiles
    IC = cfg["IC"]              # QKV token chunk (512)
    IC2 = cfg["IC2"]            # out-proj token chunk (512)
    QB = 512                    # attention q block
    KB = 128                    # attention k block
    NQT = S // QB               # q blocks per batch
    SKT = S // KB               # k tiles per batch
    NKTILES = NI // KB          # total k tiles (both batches)
    CPB = S // IC               # QKV chunks per batch
    P5C = S // IC2              # out-proj chunks per batch
    scale = 1.0 / float(np.sqrt(HD))

    nc = bacc.Bacc("TRN2", target_bir_lowering=False, debug=False,
                   num_devices=NCORES)

    xT = nc.dram_tensor("xT", [E, NI], bf16, kind="ExternalInput").ap()
    wqT = nc.dram_tensor("wqT", [E, QH], bf16, kind="ExternalInput").ap()
    wkvT = nc.dram_tensor("wkvT", [E, KVD], bf16, kind="ExternalInput").ap()
    woT = nc.dram_tensor("woT", [E, QH], bf16, kind="ExternalInput").ap()
    cosT = nc.dram_tensor("cosT", [128, S], bf16, kind="ExternalInput").ap()
    sinT = nc.dram_tensor("sinT", [128, S], bf16, kind="ExternalInput").ap()
    outT = nc.dram_tensor("outT", [QH, NI], f32, kind="ExternalOutput").ap()

    with tile.TileContext(nc) as tc, ExitStack() as persist:
        const = persist.enter_context(tc.tile_pool(name="const", bufs=1))
        qt_pool = persist.enter_context(tc.tile_pool(name="qt", bufs=1))
        kt_pool = persist.enter_context(tc.tile_pool(name="kt", bufs=1))
        vaug_pool = persist.enter_context(tc.tile_pool(name="vaug", bufs=1))
        ctxsb_pool = persist.enter_context(tc.tile_pool(name="ctxsb", bufs=1))
        stream_pool = persist.enter_context(tc.tile_pool(name="stream", bufs=2))
        rope_pool = persist.enter_context(tc.tile_pool(name="rope", bufs=2))
        vs_pool = persist.enter_context(tc.tile_pool(name="vs", bufs=2))
        exp_pool = persist.enter_context(tc.tile_pool(name="exp", bufs=3))
        rb_pool = persist.enter_context(tc.tile_pool(name="rb", bufs=2))
        cu_pool = persist.enter_context(tc.tile_pool(name="cu", bufs=3))
        ob_pool = persist.enter_context(tc.tile_pool(name="ob", bufs=2))
        proj_ps = persist.enter_context(
            tc.tile_pool(name="proj_ps", bufs=2, space="PSUM"))
        scores_ps = persist.enter_context(
            tc.tile_pool(name="scores_ps", bufs=2, space="PSUM"))
        ctx_ps_pool = persist.enter_context(
            tc.tile_pool(name="ctx_ps", bufs=2, space="PSUM"))
        dram = persist.enter_context(
            tc.tile_pool(name="dram", bufs=1, space="DRAM"))

        # ---- constants / weights (prefetched up front)
        ident = const.tile([128, 128], bf16, name="ident", tag="ident")
        make_identity(nc, ident[:, :])
        cos_sb = const.tile([128, S], bf16, name="cos_sb", tag="cos")
        nc.sync.dma_start(cos_sb[:, :], cosT)
        sin_sb = const.tile([128, S], bf16, name="sin_sb", tag="sin")
        nc.sync.dma_start(sin_sb[:, :], sinT)
        wq_sb = const.tile([128, ET, QH], bf16, name="wq_sb", tag="wq")
        nc.sync.dma_start(wq_sb[:, :, :],
                          wqT.rearrange("(t p) o -> p t o", p=128))
        wkv_sb = const.tile([128, ET, KVD], bf16, name="wkv_sb", tag="wkv")
        nc.sync.dma_start(wkv_sb[:, :, :],
                          wkvT.rearrange("(t p) o -> p t o", p=128))
        wo_sb = const.tile([128, ET, QH], bf16, name="wo_sb", tag="wo")
        nc.sync.dma_start(wo_sb[:, :, :],
                          woT.rearrange("(t p) o -> p t o", p=128))
        # triangle mask (keep where dq >= dk), duplicated for the head pair;
        # diagonal k-tiles are trimmed to their live columns so one mask fits all
        mask_sb = const.tile([128, 2, QB], bf16, name="mask_sb", tag="mask")
        nc.gpsimd.memset(mask_sb[:, :, :], 1.0)
        for hf in range(2):
            nc.gpsimd.affine_select(
                out=mask_sb[:, hf, :], in_=mask_sb[:, hf, :],
                pattern=[[1, QB]], compare_op=mybir.AluOpType.is_ge,
                fill=0.0, base=0, channel_multiplier=-1)

        # persistent activations
        qt_sb = [qt_pool.tile([128, NI], bf16, name=f"qt{m}", tag=f"qt{m}")
                 for m in range(HPC // 2)]
        kt_sb = kt_pool.tile([128, NI], bf16, tag="ktd")  # K^T duplicated 2x
        vaug = [vaug_pool.tile([128, HD + 1], bf16, name=f"va{k}", tag=f"va{k}")
                for k in range(NKTILES)]
        for k in range(NKTILES):
            nc.gpsimd.memset(vaug[k][:, HD:HD + 1], 1.0)
        ctx_sb = [ctxsb_pool.tile([128, NI], bf16, name=f"cx{m}", tag=f"cx{m}")
                  for m in range(HPC // 2)]

        # internal DRAM for per-q-block collectives (each issued right after
        # its q-block is normalized, so out-proj chunks become ready early)
        cc_in = [dram.tile([QH, QB], bf16, name=f"cc_in{h}", tag=f"ccin{h}")
                 for h in range(NQT * B)]
        cc_out = [dram.tile([E, QB], bf16, name=f"cc_out{h}", tag=f"ccout{h}",
                            addr_space="Shared")
                  for h in range(NQT * B)]

        def rope(dst, src_ps, parts, s0, ln):
            # dst = src*cos + rot_half(src)*signed_sin ; src PSUM f32, dst bf16
            # ScalarE does the (same-partition) PSUM->SBUF cast; VectorE does
            # the +-32-partition rotate-half copies and the bf16 muls/add.
            psb = rope_pool.tile([128, IC], bf16, name="psb", tag="psb")
            qs_t = rope_pool.tile([128, IC], bf16, name="qs_t", tag="qs")
            t1_t = rope_pool.tile([128, IC], bf16, name="t1_t", tag="t1")
            nc.scalar.copy(psb[:parts, :ln], src_ps[:parts, :ln])
            for h0 in range(0, parts, 64):
                nc.vector.tensor_copy(qs_t[h0:h0 + 32, :ln],
                                      psb[h0 + 32:h0 + 64, :ln])
                nc.vector.tensor_copy(qs_t[h0 + 32:h0 + 64, :ln],
                                      psb[h0:h0 + 32, :ln])
            nc.vector.tensor_mul(t1_t[:parts, :ln], psb[:parts, :ln],
                                 cos_sb[:parts, s0:s0 + ln])
            nc.vector.tensor_mul(qs_t[:parts, :ln], qs_t[:parts, :ln],
                                 sin_sb[:parts, s0:s0 + ln])
            nc.vector.tensor_add(dst, t1_t[:parts, :ln], qs_t[:parts, :ln])

        def qkv_chunk(gch):
            """QKV projection + RoPE + V transpose for 512 tokens.
            gch: global chunk id (0..NI/IC-1)."""
            i0 = gch * IC
            s0 = i0 % S
            xt = stream_pool.tile([128, ET, IC], bf16, name="xt", tag="xt")
            nc.sync.dma_start(
                xt[:, :, :],
                xT[:, i0:i0 + IC].rearrange("(t p) i -> p t i", p=128))
            for m in range(HPC // 2):
                q_ps = proj_ps.tile([128, IC], f32, name="pps", tag="proj")
                for t in range(ET):
                    nc.tensor.matmul(
                        q_ps[:, :],
                        wq_sb[:, t, m * 128:(m + 1) * 128],
                        xt[:, t, :],
                        start=(t == 0), stop=(t == ET - 1))
                rope(qt_sb[m][:, i0:i0 + IC], q_ps, 128, s0, IC)
            kv_ps = proj_ps.tile([128, IC], f32, name="pps", tag="proj")
            for t in range(ET):
                nc.tensor.matmul(
                    kv_ps[:, :],
                    wkv_sb[:, t, :],
                    xt[:, t, :],
                    start=(t == 0), stop=(t == ET - 1))
            rope(kt_sb[0:64, i0:i0 + IC], kv_ps, 64, s0, IC)
            nc.vector.tensor_copy(kt_sb[64:128, i0:i0 + IC],
                                  kt_sb[0:64, i0:i0 + IC])
            vs = vs_pool.tile([128, IC], bf16, name="vs", tag="vs")
            nc.scalar.copy(vs[64:128, :], kv_ps[64:128, :])
            for j in range(IC // 128):
                kidx = (i0 + j * 128) // 128
                vt_ps = proj_ps.tile([128, IC], bf16, name="pps", tag="proj")
                nc.tensor.transpose(vt_ps[:, 0:64],
                                    vs[64:128, j * 128:(j + 1) * 128],
                                    ident[64:128, 64:128])
                nc.scalar.copy(vaug[kidx][:, 0:HD], vt_ps[:, 0:64])

        def attn_block(b, qt, mt):
            """Attention for head pair (2mt, 2mt+1) of q-block qt of batch b.
            The two 64-contraction score matmuls use different PE row groups
            and run concurrently."""
            tok0 = b * S + qt * QB
            ctxA = ctx_ps_pool.tile([128, QB], f32, name="ctxA", tag="ctx")
            ctxB = ctx_ps_pool.tile([128, QB], f32, name="ctxB", tag="ctx")
            nkt = (qt + 1) * (QB // KB)
            for kt in range(nkt):
                ks = b * S + kt * KB
                j = kt - qt * (QB // KB)
                # diagonal k-tiles only touch q columns >= 128*j
                w0 = 128 * j if j > 0 else 0
                s_ps = scores_ps.tile([128, 2, QB], f32, name="s_ps", tag="s")
                nc.tensor.matmul(s_ps[:, 0, w0:QB],
                                 kt_sb[0:64, ks:ks + KB],
                                 qt_sb[mt][0:64, tok0 + w0:tok0 + QB],
                                 start=True, stop=True)
                nc.tensor.matmul(s_ps[:, 1, w0:QB],
                                 kt_sb[64:128, ks:ks + KB],
                                 qt_sb[mt][64:128, tok0 + w0:tok0 + QB],
                                 start=True, stop=True)
                e_t = exp_pool.tile([128, 2, QB], bf16, name="e_t", tag="e")
                nc.scalar.activation(e_t[:, :, w0:QB], s_ps[:, :, w0:QB],
                                     Exp, scale=scale)
                if j >= 0:  # diagonal block: causal mask (both heads at once)
                    nc.vector.tensor_mul(e_t[:, :, w0:QB], e_t[:, :, w0:QB],
                                         mask_sb[:, :, 0:QB - w0])
                nc.tensor.matmul(ctxA[0:HD + 1, w0:QB], vaug[b * SKT + kt][:, :],
                                 e_t[:, 0, w0:QB],
                                 start=(kt == 0), stop=(kt == nkt - 1))
                nc.tensor.matmul(ctxB[0:HD + 1, w0:QB], vaug[b * SKT + kt][:, :],
                                 e_t[:, 1, w0:QB],
                                 start=(kt == 0), stop=(kt == nkt - 1))
            for a, ctxX in ((0, ctxA), (1, ctxB)):
                # one copy frees the PSUM bank; den row then staged to
                # partition 0 (recip_approx_fast and partition_broadcast
                # only work with base-0 APs)
                ctxu = cu_pool.tile([128, QB], bf16, name="ctxu", tag="cu")
                nc.vector.tensor_copy(ctxu[0:HD + 1, :], ctxX[0:HD + 1, :])
                den1 = rb_pool.tile([1, QB], f32, name="den1", tag="den1")
                nc.vector.tensor_copy(den1[0:1, :], ctxu[HD:HD + 1, :])
                recip = rb_pool.tile([1, QB], f32, name="recip", tag="recip")
                nc.vector.reciprocal_approx_fast(recip[0:1, :], den1[0:1, :])
                rbc = rb_pool.tile([64, QB], f32, name="rbc", tag="rb")
                nc.gpsimd.partition_broadcast(rbc[:, :], recip[0:1, :])
                nc.vector.tensor_mul(
                    ctx_sb[mt][a * 64:a * 64 + 64, tok0:tok0 + QB],
                    ctxu[0:64, :], rbc[:, :])

        def attn_qblock(b, qt):
            for mt in range(HPC // 2):
                attn_block(b, qt, mt)

        def gather_qt(b, qt):
            h = NQT * b + qt
            t0 = b * S + qt * QB
            for m in range(HPC // 2):
                nc.sync.dma_start(cc_in[h][m * 128:(m + 1) * 128, :],
                                  ctx_sb[m][:, t0:t0 + QB])
            nc.gpsimd.collective_compute(
                "AllGather", mybir.AluOpType.bypass,
                replica_groups=[list(range(NCORES))],
                ins=[cc_in[h][:, :]],
                outs=[cc_out[h][:, :]])

        def p5_chunk(b, ch):
            """Output projection for 512 tokens (q-block ch) of batch b."""
            t0 = ch * IC2
            h = NQT * b + ch
            ct = stream_pool.tile([128, ET, IC2], bf16, name="ct", tag="xt")
            nc.sync.dma_start(
                ct[:, :, :],
                cc_out[h][:, :].rearrange("(t p) i -> p t i", p=128))
            for m in range(HPC // 2):
                o_ps = proj_ps.tile([128, IC2], f32, name="ops", tag="proj")
                for t in range(ET):
                    nc.tensor.matmul(
                        o_ps[:, :],
                        wo_sb[:, t, m * 128:(m + 1) * 128],
                        ct[:, t, :],
                        start=(t == 0), stop=(t == ET - 1))
                ob = ob_pool.tile([128, IC2], f32, name="ob", tag="ob")
                nc.vector.tensor_copy(ob[:, :], o_ps[:, :])
                nc.sync.dma_start(
                    outT[m * 128:(m + 1) * 128, b * S + t0:b * S + t0 + IC2],
                    ob[:, :])

        # ================= schedule =================
        # batch-0 QKV
        for ch in range(CPB):
            qkv_chunk(ch)
        # batch-0 attention (ascending); each q-block's gather fires as soon
        # as it is normalized; batch-1 QKV chunks fill PE idle time
        qkv1_sched = [[], [0], [1, 2], [3]]
        for qt in range(NQT):
            attn_qblock(0, qt)
            gather_qt(0, qt)
            for ch in qkv1_sched[qt]:
                qkv_chunk(CPB + ch)
        # batch-1 attention DESCENDING so the smallest q-block (and its
        # gather) lands last; batch-0 out-proj chunks fill PE idle time
        for i, qt in enumerate((3, 2, 1, 0)):
            attn_qblock(1, qt)
            gather_qt(1, qt)
            p5_chunk(0, i)
        # batch-1 out-proj in gather-completion order
        for qt in (3, 2, 1, 0):
            p5_chunk(1, qt)

    nc.compile()
    return nc


def make_in_maps(cfg, x, cos, sin, Wq, Wk, Wv, Wo):
    """Host-side prep: transpose/slice full inputs into per-core input maps."""
    B, S, E = cfg["B"], cfg["S"], cfg["E"]
    NH, NKV, HD, NCORES = cfg["NH"], cfg["NKV"], cfg["HD"], cfg["ncores"]
    HPC = NH // NCORES
    QH = HPC * HD
    KVPC = NKV // NCORES

    import ml_dtypes
    mmnp = ml_dtypes.bfloat16

    x = np.asarray(x, dtype=np.float32)
    cos = np.asarray(cos, dtype=np.float32)
    sin = np.asarray(sin, dtype=np.float32)
    Wq = np.asarray(Wq, dtype=np.float32)
    Wk = np.asarray(Wk, dtype=np.float32)
    Wv = np.asarray(Wv, dtype=np.float32)
    Wo = np.asarray(Wo, dtype=np.float32)

    xT = np.ascontiguousarray(x.reshape(B * S, E).T.astype(mmnp))
    cos_t = cos.T[:HD]                        # [64, S]
    cosT = np.ascontiguousarray(
        np.concatenate([cos_t, cos_t], axis=0).astype(mmnp))
    sin_t = sin.T[:HD].copy()
    sin_t[:HD // 2] *= -1.0                   # signed sin for rotate-half
    sinT = np.ascontiguousarray(
        np.concatenate([sin_t, sin_t], axis=0).astype(mmnp))

    in_maps = []
    for c in range(NCORES):
        qsl = slice(c * QH, (c + 1) * QH)
        ksl = slice(c * KVPC * HD, (c + 1) * KVPC * HD)
        wq = np.ascontiguousarray(Wq[qsl, :].T.astype(mmnp))
        wkv = np.ascontiguousarray(
            np.concatenate([Wk[ksl, :].T, Wv[ksl, :].T], axis=1).astype(mmnp))
        wo = np.ascontiguousarray(Wo[qsl, :].T.astype(mmnp))
        in_maps.append(dict(xT=xT, wqT=wq, wkvT=wkv, woT=wo,
                            cosT=cosT, sinT=sinT))
    return in_maps


def assemble_output(cfg, results):
    B, S, E = cfg["B"], cfg["S"], cfg["E"]
    outT = np.concatenate([r["outT"] for r in results], axis=0)  # [E, B*S]
    return np.ascontiguousarray(outT.T.reshape(B, S, E).astype(np.float32))


def kernel(x, mask, cos, sin, Wq, Wk, Wv, Wo):
    global LAST_RESULTS
    _ensure_concourse()
    from concourse import bass_utils

    cfg = FULL_CFG
    nc = build_gqa(cfg)
    in_maps = make_in_maps(cfg, x, cos, sin, Wq, Wk, Wv, Wo)
    res = bass_utils.run_bass_kernel_spmd(
        nc, in_maps, core_ids=list(range(cfg["ncores"])))
    LAST_RESULTS = res
    return assemble_output(cfg, res.results)
